# revision 1
# baseline (speedup 1.0000x reference)
"""MoE FeedForward kernel for 8 Trainium2 NeuronCores (v2).

Expert-parallel dispatch-by-assignment:
  - Host computes the gate (top-2 -> assign = max index, w = softmax sum) on
    jax-CPU for bit-parity with the reference's routing.
  - Tokens sorted by expert, padded to 128-token tiles, packed into 16
    single-expert slots (2 per core).  Each core holds <=2 experts' weights.
  - Host folds the LayerNorm mean into the weights: W1c = W1 - mean_f(W1),
    b1c = b1 - mean(b1), so mm1 produces centered h directly and the device
    only needs the sum of squares for the LN.
  - Device per 128-token tile: x @ W1c.T (bf16, fp32 accum, 2 PSUM halves)
    -> +b1c (DVE evac) -> sum-of-squares (ACT Square, split in halves so
    only half 1 sits on the critical chain) -> rstd via DVE bit-trick
    rsqrt + one Newton step (keeps ACT on the gelu table set the whole
    kernel) -> exact-erf GELU fused with the rstd scale (ACT) -> hybrid
    transpose (TensorE for the first two tiles, whose xbar DMA would
    queue behind the weight stream; DMA-xbar for the rest) -> bf16->fp8
    cast split across DVE/ACT -> h @ W2.T as fp8e4m3 DoubleRow matmuls
    (W2 host-scaled x512 to clear fp8 subnormals) -> per-token int8
    quantize split across DVE/ACT -> DMA out (int8 values + fp32 absmax
    scale; output bytes are the expensive part of per-exec dispatch).
  - Device returns the RAW quantized ffn output y = gelu(...) @ W2.T; the
    host dequantizes (dividing out the 512x weight scale), applies
    out = y*alpha + x*w + alpha*b2, and scatters rows back to their
    original positions.
"""

import math
import os

import numpy as np
import ml_dtypes

os.environ.setdefault("MYCRO_LOCAL_CACHE", "1")

B, S, D, F, E = 4, 2048, 1024, 2048, 8
T = B * S
NCORES = 8
PTILE = 128
LN_EPS = 1e-5
BF16 = ml_dtypes.bfloat16
MAGIC = 0x5F3759DF  # rsqrt seed

_PROG_CACHE = {}
LAST_RESULT = None
LAST_CALL = None


def _fix_waits(nc, mybir):
    """Walrus codegen rejects >1 semaphore wait per TPB instruction and ANY
    wait on a Drain (its ISA encoding has no wait slot).  Move offending
    waits onto preceding same-engine NoOps (engine queues are FIFO, so
    gating a NoOp gates the instruction)."""
    no_wait = {"Drain"}
    skip = {"UnconditionalBranch", "ConditionalBranch", "Call", "EventSemaphore"}
    work = []
    for fn in nc.m.functions:
        for blk in fn.blocks:
            for ins in blk.instructions:
                si = ins.sync_info
                waits = list(si.on_wait) if si is not None and si.on_wait else []
                op = str(ins.opcode)
                if op in skip:
                    continue
                keep = 0 if op in no_wait else 1
                if len(waits) > keep:
                    work.append((ins, waits, si, keep))
    if not work:
        return
    created = {}
    for ins, waits, si, keep in work:
        nops = []
        move = waits if keep == 0 else waits[:-1]
        for w in move:
            bi = nc.engines[ins.engine].nop(nofuse=True)
            ni = bi.ins
            ni.sync_info = mybir.SyncInfo(on_wait=[w], on_update=[])
            nops.append(ni)
        ins.sync_info = mybir.SyncInfo(
            on_wait=[] if keep == 0 else [waits[-1]],
            on_update=list(si.on_update) if si.on_update else [],
        )
        created[str(ins.name)] = nops
    nop_names = {str(n.name) for ns in created.values() for n in ns}
    for fn in nc.m.functions:
        for blk in fn.blocks:
            new_list = []
            for ins in blk.instructions:
                nm = str(ins.name)
                if nm in nop_names:
                    continue
                if nm in created:
                    new_list.extend(created[nm])
                new_list.append(ins)
            blk.instructions = new_list


def _build_program(tpc, s1, s2, general_ln, drip=True):
    from contextlib import ExitStack

    import concourse.bass as bass
    import concourse.mybir as mybir
    import concourse.tile as tile

    dt = mybir.dt
    Alu = mybir.AluOpType
    Act = mybir.ActivationFunctionType

    nc = bass.Bass()
    xtt = nc.declare_dram_parameter("xtt", [tpc, 128, D], dt.bfloat16, False)
    w1_d = nc.declare_dram_parameter("w1", [2, 128, 8 * F], dt.bfloat16, False)
    w2_d = nc.declare_dram_parameter("w2", [2, 128, 16 * D], dt.float8e4, False)
    b1_d = nc.declare_dram_parameter("b1r", [2, 128, F], dt.bfloat16, False)
    if general_ln:
        g_d = nc.declare_dram_parameter("gr", [2, 128, F], dt.bfloat16, False)
        bb_d = nc.declare_dram_parameter("br", [2, 128, F], dt.bfloat16, False)
    out_d = nc.declare_dram_parameter("out", [tpc, 128, D], dt.int8, True)
    sc_d = nc.declare_dram_parameter("sc", [tpc, 128, 1], dt.float32, True)

    with ExitStack() as ctx:
        tc = ctx.enter_context(tile.TileContext(nc))
        wp1 = ctx.enter_context(tc.tile_pool(name="w1p", bufs=2))
        wp2 = ctx.enter_context(tc.tile_pool(name="w2p", bufs=2))
        bp = ctx.enter_context(tc.tile_pool(name="b1p", bufs=2))
        xp = ctx.enter_context(tc.tile_pool(name="xp", bufs=3))
        hp = ctx.enter_context(tc.tile_pool(name="hp", bufs=2))
        jp = ctx.enter_context(tc.tile_pool(name="jp", bufs=1))
        h2p = ctx.enter_context(tc.tile_pool(name="h2p", bufs=2))
        hTp = ctx.enter_context(tc.tile_pool(name="hTp", bufs=2))
        hT8p = ctx.enter_context(tc.tile_pool(name="hT8p", bufs=2))
        fpool = ctx.enter_context(tc.tile_pool(name="fp", bufs=3))
        sp = ctx.enter_context(tc.tile_pool(name="sp", bufs=3))
        ph = ctx.enter_context(tc.tile_pool(name="ph", bufs=2, space="PSUM"))
        py = ctx.enter_context(tc.tile_pool(name="py", bufs=1, space="PSUM"))
        pt = ctx.enter_context(tc.tile_pool(name="pt", bufs=2, space="PSUM"))
        cp = ctx.enter_context(tc.tile_pool(name="cp", bufs=1))
        from concourse.masks import make_identity
        ident = cp.tile([128, 128], dt.bfloat16, tag="ident")
        make_identity(nc, ident)
        if general_ln:
            gp = ctx.enter_context(tc.tile_pool(name="gp", bufs=2))
            hnp = ctx.enter_context(tc.tile_pool(name="hnp", bufs=1))

        # Allocate weight tiles for both slots up front; DMA issue is
        # staged: enough to start tile 0 immediately, the rest drip-fed
        # two 1-MB chunks per pipeline stage so the (serially-modeled)
        # DMA queue never blocks a latency-critical x-tile load.  Weight
        # chunks ride the Pool (SWDGE) ring, small tiles ride SP, and the
        # ACT ring is reserved for the xbar transposes (DMATranspose <->
        # DMACopy transitions serialize within a ring).
        slot_tiles = {}
        for slot in range(2):
            w1t = wp1.tile([128, 8 * F], dt.bfloat16, tag="w1",
                           name=f"w1t_{slot}")
            b1t = bp.tile([128, F], dt.bfloat16, tag="b1", name=f"b1t_{slot}")
            w2t = wp2.tile([128, 16 * D], dt.float8e4, tag="w2",
                           name=f"w2t_{slot}")
            gt = bbt = None
            if general_ln:
                gt = gp.tile([128, F], dt.bfloat16, tag="g", name=f"gt_{slot}")
                bbt = gp.tile([128, F], dt.bfloat16, tag="bb",
                              name=f"bbt_{slot}")
            slot_tiles[slot] = (w1t, w2t, b1t, gt, bbt)

        def _w1_chunk(slot, q):
            w1t = slot_tiles[slot][0]
            nc.gpsimd.dma_start(w1t[:, q * 2 * F:(q + 1) * 2 * F],
                                w1_d[slot][:, q * 2 * F:(q + 1) * 2 * F])

        def _w2_chunk(slot, q):
            w2t = slot_tiles[slot][1]
            nc.gpsimd.dma_start(w2t[:, q * 4 * D:(q + 1) * 4 * D],
                                w2_d[slot][:, q * 4 * D:(q + 1) * 4 * D])

        def _b1_load(slot):
            nc.sync.dma_start(slot_tiles[slot][2], b1_d[slot])
            if general_ln:
                nc.gpsimd.dma_start(slot_tiles[slot][3], g_d[slot])
                nc.gpsimd.dma_start(slot_tiles[slot][4], bb_d[slot])

        # front block: everything tile 0's stage A+B needs soon
        xt_tiles = {}
        _w1_chunk(0, 0)
        xt_tiles[0] = xp.tile([128, 8 * 128], dt.bfloat16, tag="xt",
                              name="xt_0")
        nc.sync.dma_start(xt_tiles[0], xtt[0])
        _b1_load(0)
        _w1_chunk(0, 1)
        if tpc > 1:
            xt_tiles[1] = xp.tile([128, 8 * 128], dt.bfloat16, tag="xt",
                                  name="xt_1")
            nc.sync.dma_start(xt_tiles[1], xtt[1])
        _w1_chunk(0, 2)
        _w1_chunk(0, 3)
        _w2_chunk(0, 0)
        _w2_chunk(0, 1)

        # drip-fed remainder, two chunks per stage_a
        pending = [(_w2_chunk, 0, 2), (_w2_chunk, 0, 3),
                   (_w1_chunk, 1, 0), (_w1_chunk, 1, 1),
                   (_w1_chunk, 1, 2), (_w1_chunk, 1, 3),
                   (_b1_load, 1, None),
                   (_w2_chunk, 1, 0), (_w2_chunk, 1, 1),
                   (_w2_chunk, 1, 2), (_w2_chunk, 1, 3)]

        if not drip:
            while pending:
                fn_, sl_, q_ = pending.pop(0)
                fn_(sl_) if q_ is None else fn_(sl_, q_)

        tiles = ([(0, tl, tl) for tl in range(s1)]
                 + [(1, tl, s1 + tl) for tl in range(s2)])

        def stage_a(slot, tl, tg):
            """mm1 + LN + gelu + xbar transpose -> returns hT tile."""
            w1t, w2t, b1t, gt, bbt = slot_tiles[slot]
            # prefetch x two tiles ahead; drip two weight chunks
            if tg + 2 < tpc:
                xt_tiles[tg + 2] = xp.tile([128, 8 * 128], dt.bfloat16,
                                           tag="xt", name=f"xt_{tg+2}")
                nc.sync.dma_start(xt_tiles[tg + 2], xtt[tg + 2])
            for _ in range(2):
                if pending:
                    fn_, sl_, q_ = pending.pop(0)
                    if q_ is None:
                        fn_(sl_)
                    else:
                        fn_(sl_, q_)
            xt = xt_tiles.pop(tg)

            # ---- matmul1 (2 PSUM halves): h = x @ W1c.T + b1c ----
            h1 = hp.tile([128, F], dt.bfloat16, tag="h1", name=f"h1_{tg}")
            for half in range(2):
                hps = ph.tile([128, 1024], dt.float32, tag="hps",
                              name=f"hps_{tg}_{half}")
                for d in range(8):
                    lhsT = xt[:, d * 128:(d + 1) * 128]
                    for fb in range(2):
                        fo = half * 1024 + fb * 512
                        nc.tensor.matmul(
                            hps[:, fb * 512:(fb + 1) * 512],
                            lhsT=lhsT,
                            rhs=w1t[:, d * F + fo: d * F + fo + 512],
                            start=(d == 0),
                            stop=(d == 7),
                        )
                nc.vector.scalar_tensor_tensor(
                    out=h1[:, half * 1024:(half + 1) * 1024],
                    in0=hps, scalar=0.0,
                    in1=b1t[:, half * 1024:(half + 1) * 1024],
                    op0=Alu.add, op1=Alu.add,
                )

            # ---- sum of squares (ACT Square by halves: half 0 runs
            #      during mm1/evac of half 1, only half 1 is on the
            #      critical chain; junk large out) ----
            junk = jp.tile([128, F], dt.bfloat16, tag="junk", name=f"junk_{tg}")
            s2a = sp.tile([128, 1], dt.float32, tag="s2a", name=f"s2a_{tg}")
            s2b = sp.tile([128, 1], dt.float32, tag="s2b", name=f"s2b_{tg}")
            nc.scalar.activation(out=junk[:, 0:1024], in_=h1[:, 0:1024],
                                 func=Act.Square, accum_out=s2a)
            nc.scalar.activation(out=junk[:, 1024:2048], in_=h1[:, 1024:2048],
                                 func=Act.Square, accum_out=s2b)

            # ---- rstd = 1/sqrt(s2/F + eps) on DVE (bit-trick + Newton;
            #      keeps ACT on the gelu table set all kernel) ----
            s2t = sp.tile([128, 1], dt.float32, tag="s2", name=f"s2_{tg}")
            nc.vector.tensor_tensor(out=s2t, in0=s2a, in1=s2b, op=Alu.add)
            v = sp.tile([128, 1], dt.float32, tag="v", name=f"v_{tg}")
            nc.vector.tensor_scalar(out=v, in0=s2t, scalar1=1.0 / F,
                                    scalar2=LN_EPS, op0=Alu.mult, op1=Alu.add)
            yi = sp.tile([128, 1], dt.int32, tag="yi", name=f"yi_{tg}")
            nc.vector.tensor_scalar(out=yi, in0=v.bitcast(dt.int32),
                                    scalar1=1, scalar2=None,
                                    op0=Alu.logical_shift_right)
            y0i = sp.tile([128, 1], dt.int32, tag="y0i", name=f"y0i_{tg}")
            nc.vector.tensor_scalar(out=y0i, in0=yi, scalar1=MAGIC,
                                    scalar2=-1, op0=Alu.subtract, op1=Alu.mult)
            yk = y0i.bitcast(dt.float32)
            for it in range(1):
                t1 = sp.tile([128, 1], dt.float32, tag="t1", name=f"t1_{tg}_{it}")
                nc.vector.tensor_tensor(out=t1, in0=yk, in1=yk, op=Alu.mult)
                t2 = sp.tile([128, 1], dt.float32, tag="t2", name=f"t2_{tg}_{it}")
                nc.vector.tensor_tensor(out=t2, in0=t1, in1=v, op=Alu.mult)
                t3 = sp.tile([128, 1], dt.float32, tag="t3", name=f"t3_{tg}_{it}")
                nc.vector.tensor_scalar(out=t3, in0=t2, scalar1=-0.5,
                                        scalar2=1.5, op0=Alu.mult, op1=Alu.add)
                yn = sp.tile([128, 1], dt.float32, tag="yn", name=f"yn_{tg}_{it}")
                nc.vector.tensor_tensor(out=yn, in0=yk, in1=t3, op=Alu.mult)
                yk = yn

            # ---- gelu (+ rstd scale fused); general_ln applies g/b ----
            h2 = h2p.tile([128, F], dt.bfloat16, tag="h2", name=f"h2_{tg}")
            if not general_ln:
                nc.scalar.activation(out=h2, in_=h1, func=Act.Gelu, scale=yk)
            else:
                hn = hnp.tile([128, F], dt.bfloat16, tag="hn", name=f"hn_{tg}")
                nc.scalar.activation(out=hn, in_=h1, func=Act.Identity,
                                     scale=yk)
                hn2 = hnp.tile([128, F], dt.bfloat16, tag="hn2", name=f"hn2_{tg}")
                nc.vector.scalar_tensor_tensor(
                    out=hn2, in0=hn, scalar=0.0, in1=gt,
                    op0=Alu.add, op1=Alu.mult,
                )
                hn3 = hnp.tile([128, F], dt.bfloat16, tag="hn3", name=f"hn3_{tg}")
                nc.vector.scalar_tensor_tensor(
                    out=hn3, in0=hn2, scalar=0.0, in1=bbt,
                    op0=Alu.add, op1=Alu.add,
                )
                nc.scalar.activation(out=h2, in_=hn3, func=Act.Gelu)

            # ---- transpose h2 -> hT via DMA xbar (ACT HWDGE ring) ----
            hT = hTp.tile([128, 16, 128], dt.bfloat16, tag="hT", name=f"hT_{tg}")
            if tg < 2:
                for f in range(16):
                    ptile = pt.tile([128, 128], dt.bfloat16, tag="pt",
                                    name=f"pt_{tg}_{f}")
                    nc.tensor.transpose(ptile, h2[:, f * 128:(f + 1) * 128],
                                        ident)
                    if f % 2 == 0:
                        nc.vector.tensor_copy(hT[:, f, :], ptile)
                    else:
                        nc.scalar.copy(hT[:, f, :], ptile)
            else:
                nc.scalar.dma_start_transpose(hT, h2)
            return hT

        def stage_b(slot, tl, tg, hT):
            """mm2 + int8 quantize + DMA out."""
            w1t, w2t, b1t, gt, bbt = slot_tiles[slot]
            hT8 = hT8p.tile([128, 16, 128], dt.float8e4, tag="hT8",
                            name=f"hT8_{tg}")
            nc.vector.tensor_copy(
                hT8[:, 0:8, :].rearrange("p a b -> p (a b)"),
                hT[:, 0:8, :].rearrange("p a b -> p (a b)"))
            nc.scalar.copy(
                hT8[:, 8:16, :].rearrange("p a b -> p (a b)"),
                hT[:, 8:16, :].rearrange("p a b -> p (a b)"))
            w2v = w2t.rearrange("p (c j n) -> p c j n", c=8, j=2)
            yps = py.tile([128, D], dt.float32, tag="yps", name=f"yps_{tg}")
            for cp_ in range(8):
                lhsT = hT8[:, 2 * cp_:2 * cp_ + 2, :]
                for db in range(2):
                    nc.tensor.matmul(
                        yps[:, db * 512:(db + 1) * 512],
                        lhsT=lhsT,
                        rhs=w2v[:, cp_, :, db * 512:(db + 1) * 512],
                        start=(cp_ == 0),
                        stop=(cp_ == 7),
                        perf_mode=mybir.MatmulPerfMode.DoubleRow,
                    )

            # ---- per-token int8 quantization: q = y * 127/absmax ----
            am = sp.tile([128, 1], dt.float32, tag="am", name=f"am_{tg}")
            nc.vector.tensor_reduce(out=am, in_=yps, axis=mybir.AxisListType.X,
                                    op=Alu.max, apply_absolute_value=True)
            nc.sync.dma_start(sc_d[tg], am)
            ame = sp.tile([128, 1], dt.float32, tag="ame", name=f"ame_{tg}")
            nc.vector.tensor_scalar(out=ame, in0=am, scalar1=1e-20,
                                    scalar2=None, op0=Alu.add)
            rcp = sp.tile([128, 1], dt.float32, tag="rcp", name=f"rcp_{tg}")
            nc.vector.reciprocal(rcp, ame)
            sca = sp.tile([128, 1], dt.float32, tag="sca", name=f"sca_{tg}")
            nc.vector.tensor_scalar(out=sca, in0=rcp, scalar1=127.0,
                                    scalar2=None, op0=Alu.mult)
            q = fpool.tile([128, D], dt.int8, tag="q", name=f"q_{tg}")
            nc.vector.tensor_scalar(out=q[:, 0:512], in0=yps[:, 0:512],
                                    scalar1=sca, scalar2=None, op0=Alu.mult)
            nc.scalar.activation(out=q[:, 512:1024], in_=yps[:, 512:1024],
                                 func=Act.Copy, scale=sca)
            nc.sync.dma_start(out_d[tg], q)

        # 1-tile software skew: mm1(t+1) sits ahead of mm2(t) in the PE
        # stream, so mm2's weight-stream waits overlap mm1 compute.
        prev = None
        for slot, tl, tg in tiles:
            hT = stage_a(slot, tl, tg)
            if prev is not None:
                stage_b(*prev)
            prev = (slot, tl, tg, hT)
        stage_b(*prev)

    _fix_waits(nc, mybir)
    return nc


def _gate_host(xr, Wg, bg):
    """Replicate the reference's routing math on jax-CPU for bit-parity."""
    import jax
    import jax.numpy as jnp

    cpu = jax.devices("cpu")[0]
    with jax.default_device(cpu):
        xj = jnp.asarray(xr)
        logits = xj @ jnp.asarray(Wg).T + jnp.asarray(bg)
        top_v, top_i = jax.lax.top_k(logits, 2)
        w = jnp.sum(jax.nn.softmax(top_v, axis=-1), axis=-1)
        assign = jnp.max(top_i, axis=-1)
        return np.asarray(assign), np.asarray(w, dtype=np.float32)


def _pack_slots(counts):
    """Pack per-expert tile demands into 16 single-expert slots (8 of size
    s1, 8 of size s2, s1+s2 = tpc), minimizing tpc via DP."""
    demands = {e: int(math.ceil(c / PTILE)) for e, c in enumerate(counts) if c > 0}
    experts = sorted(demands, key=lambda k: -demands[k])
    total = sum(demands.values())
    tpc = max(2, math.ceil(total / NCORES))
    while True:
        s1 = math.ceil(tpc / 2)
        s2 = tpc - s1
        opts = []
        for e in experts:
            d = demands[e]
            o = []
            for a in range(9):
                for b in range(9):
                    if a + b == 0:
                        continue
                    if a * s1 + b * s2 >= d:
                        if not any(a2 <= a and b2 <= b for a2, b2 in o):
                            o.append((a, b))
            o = [(a, b) for a, b in o
                 if not any((a2 <= a and b2 <= b and (a2, b2) != (a, b))
                            for a2, b2 in o)]
            opts.append(o)
        states = {(0, 0): []}
        for o in opts:
            nxt = {}
            for (ua, ub), path in states.items():
                for a, b in o:
                    k = (ua + a, ub + b)
                    if k[0] <= 8 and k[1] <= 8 and k not in nxt:
                        nxt[k] = path + [(a, b)]
            states = nxt
            if not states:
                break
        if states:
            choice = next(iter(states.values()))
            break
        tpc += 1
    g1, g2 = [], []
    for e, (a, b) in zip(experts, choice):
        rem = demands[e]
        for _ in range(a):
            g1.append({"expert": e, "size": s1, "nreal": min(rem, s1)})
            rem -= min(rem, s1)
        for _ in range(b):
            g2.append({"expert": e, "size": s2, "nreal": min(rem, s2)})
            rem -= min(rem, s2)
        assert rem == 0
    big_e = experts[0]
    while len(g1) < 8:
        g1.append({"expert": big_e, "size": s1, "nreal": 0})
    while len(g2) < 8:
        g2.append({"expert": big_e, "size": s2, "nreal": 0})
    return tpc, s1, s2, list(zip(g1, g2[::-1]))


def kernel(x, Wg, bg, W1, b1, ln_g, ln_b, W2, b2, res_scale):
    global LAST_RESULT, LAST_CALL
    x = np.asarray(x, dtype=np.float32)
    Wg = np.asarray(Wg, dtype=np.float32)
    bg = np.asarray(bg, dtype=np.float32)
    W1 = np.asarray(W1, dtype=np.float32)
    b1 = np.asarray(b1, dtype=np.float32)
    ln_g = np.asarray(ln_g, dtype=np.float32)
    ln_b = np.asarray(ln_b, dtype=np.float32)
    W2 = np.asarray(W2, dtype=np.float32)
    b2 = np.asarray(b2, dtype=np.float32)
    res_scale = np.asarray(res_scale, dtype=np.float32)

    xr = x.reshape(T, D)
    assign, w = _gate_host(xr, Wg, bg)

    counts = np.bincount(assign, minlength=E)
    order = np.argsort(assign, kind="stable")
    tpc, s1, s2, core_slots = _pack_slots(counts)
    general_ln = not (np.all(ln_g == 1.0) and np.all(ln_b == 0.0))

    starts = np.zeros(E + 1, np.int64)
    np.cumsum(counts, out=starts[1:])
    exp_tiles = {}
    for e in range(E):
        c = int(counts[e])
        if c == 0:
            continue
        toks = order[starts[e]:starts[e] + c]
        ntl = math.ceil(c / PTILE)
        padded = np.concatenate([toks, np.repeat(toks[-1], ntl * PTILE - c)])
        valid = np.zeros(ntl * PTILE, bool)
        valid[:c] = True
        exp_tiles[e] = (padded.reshape(ntl, PTILE), valid.reshape(ntl, PTILE))
    cursor = {e: 0 for e in exp_tiles}

    # centered weights: h - mean_f(h) == x @ W1c.T + b1c
    w1bar = W1.mean(axis=1)          # [E, D]
    b1bar = b1.mean(axis=1)          # [E]
    used = sorted({s["expert"] for pair in core_slots for s in pair})
    W1P, W2P, B1R, GR, BR = {}, {}, {}, {}, {}
    for e in used:
        W1c = W1[e] - w1bar[e][None, :]
        b1c = b1[e] - b1bar[e]
        W1P[e] = np.ascontiguousarray(
            W1c.T.reshape(8, 128, F).transpose(1, 0, 2).reshape(128, 8 * F)
        ).astype(BF16)
        # DoubleRow layout: col = cpair*2048 + j*1024 + d, value W2[d, f]
        # with f = (2*cpair + j)*128 + p, scaled x512 to clear fp8e4m3
        # subnormals (the host dequant divides it back out).
        W2P[e] = np.ascontiguousarray(
            (W2[e].T * 512.0).reshape(8, 2, 128, D).transpose(2, 0, 1, 3)
            .reshape(128, 16 * D)
        ).astype(ml_dtypes.float8_e4m3fn)
        B1R[e] = np.broadcast_to(b1c, (128, F)).astype(BF16)
        if general_ln:
            GR[e] = np.broadcast_to(ln_g[e], (128, F)).astype(BF16)
            BR[e] = np.broadcast_to(ln_b[e], (128, F)).astype(BF16)

    in_maps = []
    scatter = []  # per core: (token_ids, valid, expert_row)
    for slot_a, slot_b in core_slots:
        tok_ids = np.zeros((tpc, PTILE), np.int64)
        valid = np.zeros((tpc, PTILE), bool)
        e_tile = np.zeros(tpc, np.int64)
        ti = 0
        for slot, size in ((slot_a, s1), (slot_b, s2)):
            e = slot["expert"]
            tiles, vmask = exp_tiles.get(e, (None, None))
            for k in range(size):
                if k < slot["nreal"]:
                    idx = cursor[e]
                    cursor[e] += 1
                    tok_ids[ti] = tiles[idx]
                    valid[ti] = vmask[idx]
                else:
                    tok_ids[ti] = tiles[0] if tiles is not None else 0
                    valid[ti] = False
                e_tile[ti] = e
                ti += 1
        ids = tok_ids.reshape(-1)
        xg = xr[ids]  # [tpc*128, D]
        xtt = (
            xg.reshape(tpc, PTILE, 8, 128)
            .transpose(0, 3, 2, 1)
            .reshape(tpc, 128, 8 * 128)
        ).astype(BF16)
        im = {
            "xtt": np.ascontiguousarray(xtt),
            "w1": np.stack([W1P[slot_a["expert"]], W1P[slot_b["expert"]]]),
            "w2": np.stack([W2P[slot_a["expert"]], W2P[slot_b["expert"]]]),
            "b1r": np.stack([B1R[slot_a["expert"]], B1R[slot_b["expert"]]]),
        }
        if general_ln:
            im["gr"] = np.stack([GR[slot_a["expert"]], GR[slot_b["expert"]]])
            im["br"] = np.stack([BR[slot_a["expert"]], BR[slot_b["expert"]]])
        in_maps.append(im)
        scatter.append((ids, valid.reshape(-1), np.repeat(e_tile, PTILE)))

    key = (tpc, s1, s2, general_ln)
    if key not in _PROG_CACHE:
        _PROG_CACHE[key] = _build_program(*key)
    nc = _PROG_CACHE[key]

    from concourse.bass_utils import run_bass_kernel_spmd

    LAST_CALL = (nc, in_maps)
    res = run_bass_kernel_spmd(nc, in_maps, core_ids=list(range(NCORES)))
    LAST_RESULT = res

    out = np.zeros((T, D), np.float32)
    covered = 0
    for core in range(NCORES):
        q = np.asarray(res.results[core]["out"]).reshape(
            tpc * PTILE, D).astype(np.float32)
        am = np.asarray(res.results[core]["sc"]).reshape(
            tpc * PTILE, 1).astype(np.float32)
        y = q * ((am + 1e-20) / (127.0 * 512.0))
        ids, valid, e_row = scatter[core]
        idv = ids[valid]
        ev = e_row[valid]
        wv = w[idv]
        alpha = res_scale[ev] * wv
        out[idv] = (y[valid] * alpha[:, None]
                    + xr[idv] * wv[:, None]
                    + alpha[:, None] * b2[ev])
        covered += int(valid.sum())
    assert covered == T, f"coverage {covered} != {T}"
    return out.reshape(B, S, D)



# revision 2
# speedup vs baseline: 11.1314x; 11.1314x over previous
"""MoE FeedForward kernel for 8 Trainium2 NeuronCores (v2).

Expert-parallel dispatch-by-assignment:
  - Host computes the gate (top-2 -> assign = max index, w = softmax sum) on
    jax-CPU for bit-parity with the reference's routing.
  - Tokens sorted by expert, padded to 128-token tiles, packed into 16
    single-expert slots (2 per core).  Each core holds <=2 experts' weights.
  - Host folds the LayerNorm mean into the weights: W1c = W1 - mean_f(W1),
    b1c = b1 - mean(b1), so mm1 produces centered h directly and the device
    only needs the sum of squares for the LN.
  - Device per 128-token tile: x @ W1c.T (bf16, fp32 accum, 2 PSUM halves)
    -> +b1c (DVE evac) -> sum-of-squares (ACT Square, split in halves so
    only half 1 sits on the critical chain) -> rstd via DVE bit-trick
    rsqrt + one Newton step (keeps ACT on the gelu table set the whole
    kernel) -> exact-erf GELU fused with the rstd scale (ACT) -> hybrid
    transpose (TensorE for the first two tiles, whose xbar DMA would
    queue behind the weight stream; DMA-xbar for the rest) -> bf16->fp8
    cast split across DVE/ACT -> h @ W2.T as fp8e4m3 DoubleRow matmuls
    (W2 host-scaled x512 to clear fp8 subnormals) -> per-token int8
    quantize split across DVE/ACT -> DMA out (int8 values + fp32 absmax
    scale; output bytes are the expensive part of per-exec dispatch).
  - Device returns the RAW quantized ffn output y = gelu(...) @ W2.T; the
    host dequantizes (dividing out the 512x weight scale), applies
    out = y*alpha + x*w + alpha*b2, and scatters rows back to their
    original positions.
"""

import math
import os

import numpy as np
import ml_dtypes

os.environ.setdefault("MYCRO_LOCAL_CACHE", "1")

B, S, D, F, E = 4, 2048, 1024, 2048, 8
T = B * S
NCORES = 8
PTILE = 128
LN_EPS = 1e-5
BF16 = ml_dtypes.bfloat16
MAGIC = 0x5F3759DF  # rsqrt seed

_PROG_CACHE = {}
LAST_RESULT = None
LAST_CALL = None


def _fix_waits(nc, mybir):
    """Walrus codegen rejects >1 semaphore wait per TPB instruction and ANY
    wait on a Drain (its ISA encoding has no wait slot).  Move offending
    waits onto preceding same-engine NoOps (engine queues are FIFO, so
    gating a NoOp gates the instruction)."""
    no_wait = {"Drain"}
    skip = {"UnconditionalBranch", "ConditionalBranch", "Call", "EventSemaphore"}
    work = []
    for fn in nc.m.functions:
        for blk in fn.blocks:
            for ins in blk.instructions:
                si = ins.sync_info
                waits = list(si.on_wait) if si is not None and si.on_wait else []
                op = str(ins.opcode)
                if op in skip:
                    continue
                keep = 0 if op in no_wait else 1
                if len(waits) > keep:
                    work.append((ins, waits, si, keep))
    if not work:
        return
    created = {}
    for ins, waits, si, keep in work:
        nops = []
        move = waits if keep == 0 else waits[:-1]
        for w in move:
            bi = nc.engines[ins.engine].nop(nofuse=True)
            ni = bi.ins
            ni.sync_info = mybir.SyncInfo(on_wait=[w], on_update=[])
            nops.append(ni)
        ins.sync_info = mybir.SyncInfo(
            on_wait=[] if keep == 0 else [waits[-1]],
            on_update=list(si.on_update) if si.on_update else [],
        )
        created[str(ins.name)] = nops
    nop_names = {str(n.name) for ns in created.values() for n in ns}
    for fn in nc.m.functions:
        for blk in fn.blocks:
            new_list = []
            for ins in blk.instructions:
                nm = str(ins.name)
                if nm in nop_names:
                    continue
                if nm in created:
                    new_list.extend(created[nm])
                new_list.append(ins)
            blk.instructions = new_list


def _build_program(tpc, s1, s2, general_ln, drip=True, const_w=None):
    from contextlib import ExitStack

    import concourse.bass as bass
    import concourse.mybir as mybir
    import concourse.tile as tile

    dt = mybir.dt
    Alu = mybir.AluOpType
    Act = mybir.ActivationFunctionType

    nc = bass.Bass()
    xtt = nc.declare_dram_parameter("xtt", [tpc, 128, D], dt.bfloat16, False)
    if const_w is not None:
        w1_d = nc.inline_tensor(const_w["w1"], "w1c")
        w2_d = nc.inline_tensor(const_w["w2"], "w2c")
        b1_d = nc.inline_tensor(const_w["b1r"], "b1c")
    else:
        w1_d = nc.declare_dram_parameter("w1", [2, 128, 8 * F], dt.bfloat16,
                                         False)
        w2_d = nc.declare_dram_parameter("w2", [2, 128, 16 * D], dt.float8e4,
                                         False)
        b1_d = nc.declare_dram_parameter("b1r", [2, 128, F], dt.bfloat16,
                                         False)
    if general_ln:
        g_d = nc.declare_dram_parameter("gr", [2, 128, F], dt.bfloat16, False)
        bb_d = nc.declare_dram_parameter("br", [2, 128, F], dt.bfloat16, False)
    out_d = nc.declare_dram_parameter("out", [tpc, 128, D], dt.int8, True)
    sc_d = nc.declare_dram_parameter("sc", [tpc, 128, 1], dt.float32, True)

    with ExitStack() as ctx:
        tc = ctx.enter_context(tile.TileContext(nc))
        wp1 = ctx.enter_context(tc.tile_pool(name="w1p", bufs=2))
        wp2 = ctx.enter_context(tc.tile_pool(name="w2p", bufs=2))
        bp = ctx.enter_context(tc.tile_pool(name="b1p", bufs=2))
        xp = ctx.enter_context(tc.tile_pool(name="xp", bufs=3))
        hp = ctx.enter_context(tc.tile_pool(name="hp", bufs=2))
        jp = ctx.enter_context(tc.tile_pool(name="jp", bufs=1))
        h2p = ctx.enter_context(tc.tile_pool(name="h2p", bufs=2))
        hTp = ctx.enter_context(tc.tile_pool(name="hTp", bufs=2))
        hT8p = ctx.enter_context(tc.tile_pool(name="hT8p", bufs=2))
        fpool = ctx.enter_context(tc.tile_pool(name="fp", bufs=3))
        sp = ctx.enter_context(tc.tile_pool(name="sp", bufs=3))
        ph = ctx.enter_context(tc.tile_pool(name="ph", bufs=2, space="PSUM"))
        py = ctx.enter_context(tc.tile_pool(name="py", bufs=1, space="PSUM"))
        pt = ctx.enter_context(tc.tile_pool(name="pt", bufs=2, space="PSUM"))
        cp = ctx.enter_context(tc.tile_pool(name="cp", bufs=1))
        from concourse.masks import make_identity
        ident = cp.tile([128, 128], dt.bfloat16, tag="ident")
        make_identity(nc, ident)
        if general_ln:
            gp = ctx.enter_context(tc.tile_pool(name="gp", bufs=2))
            hnp = ctx.enter_context(tc.tile_pool(name="hnp", bufs=1))

        # Allocate weight tiles for both slots up front; DMA issue is
        # staged: enough to start tile 0 immediately, the rest drip-fed
        # two 1-MB chunks per pipeline stage so the (serially-modeled)
        # DMA queue never blocks a latency-critical x-tile load.  Weight
        # chunks ride the Pool (SWDGE) ring, small tiles ride SP, and the
        # ACT ring is reserved for the xbar transposes (DMATranspose <->
        # DMACopy transitions serialize within a ring).
        slot_tiles = {}
        for slot in range(2):
            w1t = wp1.tile([128, 8 * F], dt.bfloat16, tag="w1",
                           name=f"w1t_{slot}")
            b1t = bp.tile([128, F], dt.bfloat16, tag="b1", name=f"b1t_{slot}")
            w2t = wp2.tile([128, 16 * D], dt.float8e4, tag="w2",
                           name=f"w2t_{slot}")
            gt = bbt = None
            if general_ln:
                gt = gp.tile([128, F], dt.bfloat16, tag="g", name=f"gt_{slot}")
                bbt = gp.tile([128, F], dt.bfloat16, tag="bb",
                              name=f"bbt_{slot}")
            slot_tiles[slot] = (w1t, w2t, b1t, gt, bbt)

        def _w1_chunk(slot, q):
            w1t = slot_tiles[slot][0]
            nc.gpsimd.dma_start(w1t[:, q * 2 * F:(q + 1) * 2 * F],
                                w1_d[slot][:, q * 2 * F:(q + 1) * 2 * F])

        def _w2_chunk(slot, q):
            w2t = slot_tiles[slot][1]
            nc.gpsimd.dma_start(w2t[:, q * 4 * D:(q + 1) * 4 * D],
                                w2_d[slot][:, q * 4 * D:(q + 1) * 4 * D])

        def _b1_load(slot):
            nc.sync.dma_start(slot_tiles[slot][2], b1_d[slot])
            if general_ln:
                nc.gpsimd.dma_start(slot_tiles[slot][3], g_d[slot])
                nc.gpsimd.dma_start(slot_tiles[slot][4], bb_d[slot])

        # front block: everything tile 0's stage A+B needs soon
        xt_tiles = {}
        _w1_chunk(0, 0)
        xt_tiles[0] = xp.tile([128, 8 * 128], dt.bfloat16, tag="xt",
                              name="xt_0")
        nc.sync.dma_start(xt_tiles[0], xtt[0])
        _b1_load(0)
        _w1_chunk(0, 1)
        if tpc > 1:
            xt_tiles[1] = xp.tile([128, 8 * 128], dt.bfloat16, tag="xt",
                                  name="xt_1")
            nc.sync.dma_start(xt_tiles[1], xtt[1])
        _w1_chunk(0, 2)
        _w1_chunk(0, 3)
        _w2_chunk(0, 0)
        _w2_chunk(0, 1)

        # drip-fed remainder, two chunks per stage_a
        pending = [(_w2_chunk, 0, 2), (_w2_chunk, 0, 3),
                   (_w1_chunk, 1, 0), (_w1_chunk, 1, 1),
                   (_w1_chunk, 1, 2), (_w1_chunk, 1, 3),
                   (_b1_load, 1, None),
                   (_w2_chunk, 1, 0), (_w2_chunk, 1, 1),
                   (_w2_chunk, 1, 2), (_w2_chunk, 1, 3)]

        if not drip:
            while pending:
                fn_, sl_, q_ = pending.pop(0)
                fn_(sl_) if q_ is None else fn_(sl_, q_)

        tiles = ([(0, tl, tl) for tl in range(s1)]
                 + [(1, tl, s1 + tl) for tl in range(s2)])

        def stage_a(slot, tl, tg):
            """mm1 + LN + gelu + xbar transpose -> returns hT tile."""
            w1t, w2t, b1t, gt, bbt = slot_tiles[slot]
            # prefetch x two tiles ahead; drip two weight chunks
            if tg + 2 < tpc:
                xt_tiles[tg + 2] = xp.tile([128, 8 * 128], dt.bfloat16,
                                           tag="xt", name=f"xt_{tg+2}")
                nc.sync.dma_start(xt_tiles[tg + 2], xtt[tg + 2])
            for _ in range(2):
                if pending:
                    fn_, sl_, q_ = pending.pop(0)
                    if q_ is None:
                        fn_(sl_)
                    else:
                        fn_(sl_, q_)
            xt = xt_tiles.pop(tg)

            # ---- matmul1 (2 PSUM halves): h = x @ W1c.T + b1c ----
            h1 = hp.tile([128, F], dt.bfloat16, tag="h1", name=f"h1_{tg}")
            for half in range(2):
                hps = ph.tile([128, 1024], dt.float32, tag="hps",
                              name=f"hps_{tg}_{half}")
                for d in range(8):
                    lhsT = xt[:, d * 128:(d + 1) * 128]
                    for fb in range(2):
                        fo = half * 1024 + fb * 512
                        nc.tensor.matmul(
                            hps[:, fb * 512:(fb + 1) * 512],
                            lhsT=lhsT,
                            rhs=w1t[:, d * F + fo: d * F + fo + 512],
                            start=(d == 0),
                            stop=(d == 7),
                        )
                nc.vector.scalar_tensor_tensor(
                    out=h1[:, half * 1024:(half + 1) * 1024],
                    in0=hps, scalar=0.0,
                    in1=b1t[:, half * 1024:(half + 1) * 1024],
                    op0=Alu.add, op1=Alu.add,
                )

            # ---- sum of squares (ACT Square by halves: half 0 runs
            #      during mm1/evac of half 1, only half 1 is on the
            #      critical chain; junk large out) ----
            junk = jp.tile([128, F], dt.bfloat16, tag="junk", name=f"junk_{tg}")
            s2a = sp.tile([128, 1], dt.float32, tag="s2a", name=f"s2a_{tg}")
            s2b = sp.tile([128, 1], dt.float32, tag="s2b", name=f"s2b_{tg}")
            nc.scalar.activation(out=junk[:, 0:1024], in_=h1[:, 0:1024],
                                 func=Act.Square, accum_out=s2a)
            nc.scalar.activation(out=junk[:, 1024:2048], in_=h1[:, 1024:2048],
                                 func=Act.Square, accum_out=s2b)

            # ---- rstd = 1/sqrt(s2/F + eps) on DVE (bit-trick + Newton;
            #      keeps ACT on the gelu table set all kernel) ----
            s2t = sp.tile([128, 1], dt.float32, tag="s2", name=f"s2_{tg}")
            nc.vector.tensor_tensor(out=s2t, in0=s2a, in1=s2b, op=Alu.add)
            v = sp.tile([128, 1], dt.float32, tag="v", name=f"v_{tg}")
            nc.vector.tensor_scalar(out=v, in0=s2t, scalar1=1.0 / F,
                                    scalar2=LN_EPS, op0=Alu.mult, op1=Alu.add)
            yi = sp.tile([128, 1], dt.int32, tag="yi", name=f"yi_{tg}")
            nc.vector.tensor_scalar(out=yi, in0=v.bitcast(dt.int32),
                                    scalar1=1, scalar2=None,
                                    op0=Alu.logical_shift_right)
            y0i = sp.tile([128, 1], dt.int32, tag="y0i", name=f"y0i_{tg}")
            nc.vector.tensor_scalar(out=y0i, in0=yi, scalar1=MAGIC,
                                    scalar2=-1, op0=Alu.subtract, op1=Alu.mult)
            yk = y0i.bitcast(dt.float32)
            for it in range(1):
                t1 = sp.tile([128, 1], dt.float32, tag="t1", name=f"t1_{tg}_{it}")
                nc.vector.tensor_tensor(out=t1, in0=yk, in1=yk, op=Alu.mult)
                t2 = sp.tile([128, 1], dt.float32, tag="t2", name=f"t2_{tg}_{it}")
                nc.vector.tensor_tensor(out=t2, in0=t1, in1=v, op=Alu.mult)
                t3 = sp.tile([128, 1], dt.float32, tag="t3", name=f"t3_{tg}_{it}")
                nc.vector.tensor_scalar(out=t3, in0=t2, scalar1=-0.5,
                                        scalar2=1.5, op0=Alu.mult, op1=Alu.add)
                yn = sp.tile([128, 1], dt.float32, tag="yn", name=f"yn_{tg}_{it}")
                nc.vector.tensor_tensor(out=yn, in0=yk, in1=t3, op=Alu.mult)
                yk = yn

            # ---- gelu (+ rstd scale fused); general_ln applies g/b ----
            h2 = h2p.tile([128, F], dt.bfloat16, tag="h2", name=f"h2_{tg}")
            if not general_ln:
                nc.scalar.activation(out=h2, in_=h1, func=Act.Gelu, scale=yk)
            else:
                hn = hnp.tile([128, F], dt.bfloat16, tag="hn", name=f"hn_{tg}")
                nc.scalar.activation(out=hn, in_=h1, func=Act.Identity,
                                     scale=yk)
                hn2 = hnp.tile([128, F], dt.bfloat16, tag="hn2", name=f"hn2_{tg}")
                nc.vector.scalar_tensor_tensor(
                    out=hn2, in0=hn, scalar=0.0, in1=gt,
                    op0=Alu.add, op1=Alu.mult,
                )
                hn3 = hnp.tile([128, F], dt.bfloat16, tag="hn3", name=f"hn3_{tg}")
                nc.vector.scalar_tensor_tensor(
                    out=hn3, in0=hn2, scalar=0.0, in1=bbt,
                    op0=Alu.add, op1=Alu.add,
                )
                nc.scalar.activation(out=h2, in_=hn3, func=Act.Gelu)

            # ---- transpose h2 -> hT via DMA xbar (ACT HWDGE ring) ----
            hT = hTp.tile([128, 16, 128], dt.bfloat16, tag="hT", name=f"hT_{tg}")
            if tg < 2:
                for f in range(16):
                    ptile = pt.tile([128, 128], dt.bfloat16, tag="pt",
                                    name=f"pt_{tg}_{f}")
                    nc.tensor.transpose(ptile, h2[:, f * 128:(f + 1) * 128],
                                        ident)
                    if f % 2 == 0:
                        nc.vector.tensor_copy(hT[:, f, :], ptile)
                    else:
                        nc.scalar.copy(hT[:, f, :], ptile)
            else:
                nc.scalar.dma_start_transpose(hT, h2)
            return hT

        def stage_b(slot, tl, tg, hT):
            """mm2 + int8 quantize + DMA out."""
            w1t, w2t, b1t, gt, bbt = slot_tiles[slot]
            hT8 = hT8p.tile([128, 16, 128], dt.float8e4, tag="hT8",
                            name=f"hT8_{tg}")
            nc.vector.tensor_copy(
                hT8[:, 0:8, :].rearrange("p a b -> p (a b)"),
                hT[:, 0:8, :].rearrange("p a b -> p (a b)"))
            nc.scalar.copy(
                hT8[:, 8:16, :].rearrange("p a b -> p (a b)"),
                hT[:, 8:16, :].rearrange("p a b -> p (a b)"))
            w2v = w2t.rearrange("p (c j n) -> p c j n", c=8, j=2)
            yps = py.tile([128, D], dt.float32, tag="yps", name=f"yps_{tg}")
            for cp_ in range(8):
                lhsT = hT8[:, 2 * cp_:2 * cp_ + 2, :]
                for db in range(2):
                    nc.tensor.matmul(
                        yps[:, db * 512:(db + 1) * 512],
                        lhsT=lhsT,
                        rhs=w2v[:, cp_, :, db * 512:(db + 1) * 512],
                        start=(cp_ == 0),
                        stop=(cp_ == 7),
                        perf_mode=mybir.MatmulPerfMode.DoubleRow,
                    )

            # ---- per-token int8 quantization: q = y * 127/absmax ----
            am = sp.tile([128, 1], dt.float32, tag="am", name=f"am_{tg}")
            nc.vector.tensor_reduce(out=am, in_=yps, axis=mybir.AxisListType.X,
                                    op=Alu.max, apply_absolute_value=True)
            nc.sync.dma_start(sc_d[tg], am)
            ame = sp.tile([128, 1], dt.float32, tag="ame", name=f"ame_{tg}")
            nc.vector.tensor_scalar(out=ame, in0=am, scalar1=1e-20,
                                    scalar2=None, op0=Alu.add)
            rcp = sp.tile([128, 1], dt.float32, tag="rcp", name=f"rcp_{tg}")
            nc.vector.reciprocal(rcp, ame)
            sca = sp.tile([128, 1], dt.float32, tag="sca", name=f"sca_{tg}")
            nc.vector.tensor_scalar(out=sca, in0=rcp, scalar1=127.0,
                                    scalar2=None, op0=Alu.mult)
            q = fpool.tile([128, D], dt.int8, tag="q", name=f"q_{tg}")
            nc.vector.tensor_scalar(out=q[:, 0:512], in0=yps[:, 0:512],
                                    scalar1=sca, scalar2=None, op0=Alu.mult)
            nc.scalar.activation(out=q[:, 512:1024], in_=yps[:, 512:1024],
                                 func=Act.Copy, scale=sca)
            nc.sync.dma_start(out_d[tg], q)

        # 1-tile software skew: mm1(t+1) sits ahead of mm2(t) in the PE
        # stream, so mm2's weight-stream waits overlap mm1 compute.
        prev = None
        for slot, tl, tg in tiles:
            hT = stage_a(slot, tl, tg)
            if prev is not None:
                stage_b(*prev)
            prev = (slot, tl, tg, hT)
        stage_b(*prev)

    _fix_waits(nc, mybir)
    return nc


def _gate_host(xr, Wg, bg):
    """Replicate the reference's routing math on jax-CPU for bit-parity."""
    import jax
    import jax.numpy as jnp

    cpu = jax.devices("cpu")[0]
    with jax.default_device(cpu):
        xj = jnp.asarray(xr)
        logits = xj @ jnp.asarray(Wg).T + jnp.asarray(bg)
        top_v, top_i = jax.lax.top_k(logits, 2)
        w = jnp.sum(jax.nn.softmax(top_v, axis=-1), axis=-1)
        assign = jnp.max(top_i, axis=-1)
        return np.asarray(assign), np.asarray(w, dtype=np.float32)


def _pack_slots(counts):
    """Pack per-expert tile demands into 16 single-expert slots (8 of size
    s1, 8 of size s2, s1+s2 = tpc), minimizing tpc via DP."""
    demands = {e: int(math.ceil(c / PTILE)) for e, c in enumerate(counts) if c > 0}
    experts = sorted(demands, key=lambda k: -demands[k])
    total = sum(demands.values())
    tpc = max(2, math.ceil(total / NCORES))
    while True:
        s1 = math.ceil(tpc / 2)
        s2 = tpc - s1
        opts = []
        for e in experts:
            d = demands[e]
            o = []
            for a in range(9):
                for b in range(9):
                    if a + b == 0:
                        continue
                    if a * s1 + b * s2 >= d:
                        if not any(a2 <= a and b2 <= b for a2, b2 in o):
                            o.append((a, b))
            o = [(a, b) for a, b in o
                 if not any((a2 <= a and b2 <= b and (a2, b2) != (a, b))
                            for a2, b2 in o)]
            opts.append(o)
        states = {(0, 0): []}
        for o in opts:
            nxt = {}
            for (ua, ub), path in states.items():
                for a, b in o:
                    k = (ua + a, ub + b)
                    if k[0] <= 8 and k[1] <= 8 and k not in nxt:
                        nxt[k] = path + [(a, b)]
            states = nxt
            if not states:
                break
        if states:
            choice = next(iter(states.values()))
            break
        tpc += 1
    g1, g2 = [], []
    for e, (a, b) in zip(experts, choice):
        rem = demands[e]
        for _ in range(a):
            g1.append({"expert": e, "size": s1, "nreal": min(rem, s1)})
            rem -= min(rem, s1)
        for _ in range(b):
            g2.append({"expert": e, "size": s2, "nreal": min(rem, s2)})
            rem -= min(rem, s2)
        assert rem == 0
    big_e = experts[0]
    while len(g1) < 8:
        g1.append({"expert": big_e, "size": s1, "nreal": 0})
    while len(g2) < 8:
        g2.append({"expert": big_e, "size": s2, "nreal": 0})
    return tpc, s1, s2, list(zip(g1, g2[::-1]))


def kernel(x, Wg, bg, W1, b1, ln_g, ln_b, W2, b2, res_scale):
    global LAST_RESULT, LAST_CALL
    x = np.asarray(x, dtype=np.float32)
    Wg = np.asarray(Wg, dtype=np.float32)
    bg = np.asarray(bg, dtype=np.float32)
    W1 = np.asarray(W1, dtype=np.float32)
    b1 = np.asarray(b1, dtype=np.float32)
    ln_g = np.asarray(ln_g, dtype=np.float32)
    ln_b = np.asarray(ln_b, dtype=np.float32)
    W2 = np.asarray(W2, dtype=np.float32)
    b2 = np.asarray(b2, dtype=np.float32)
    res_scale = np.asarray(res_scale, dtype=np.float32)

    xr = x.reshape(T, D)
    assign, w = _gate_host(xr, Wg, bg)

    counts = np.bincount(assign, minlength=E)
    order = np.argsort(assign, kind="stable")
    tpc, s1, s2, core_slots = _pack_slots(counts)
    general_ln = not (np.all(ln_g == 1.0) and np.all(ln_b == 0.0))

    starts = np.zeros(E + 1, np.int64)
    np.cumsum(counts, out=starts[1:])
    exp_tiles = {}
    for e in range(E):
        c = int(counts[e])
        if c == 0:
            continue
        toks = order[starts[e]:starts[e] + c]
        ntl = math.ceil(c / PTILE)
        padded = np.concatenate([toks, np.repeat(toks[-1], ntl * PTILE - c)])
        valid = np.zeros(ntl * PTILE, bool)
        valid[:c] = True
        exp_tiles[e] = (padded.reshape(ntl, PTILE), valid.reshape(ntl, PTILE))
    cursor = {e: 0 for e in exp_tiles}

    # centered weights: h - mean_f(h) == x @ W1c.T + b1c
    w1bar = W1.mean(axis=1)          # [E, D]
    b1bar = b1.mean(axis=1)          # [E]
    used = sorted({s["expert"] for pair in core_slots for s in pair})
    W1P, W2P, B1R, GR, BR = {}, {}, {}, {}, {}
    for e in used:
        W1c = W1[e] - w1bar[e][None, :]
        b1c = b1[e] - b1bar[e]
        W1P[e] = np.ascontiguousarray(
            W1c.T.reshape(8, 128, F).transpose(1, 0, 2).reshape(128, 8 * F)
        ).astype(BF16)
        # DoubleRow layout: col = cpair*2048 + j*1024 + d, value W2[d, f]
        # with f = (2*cpair + j)*128 + p, scaled x512 to clear fp8e4m3
        # subnormals (the host dequant divides it back out).
        W2P[e] = np.ascontiguousarray(
            (W2[e].T * 512.0).reshape(8, 2, 128, D).transpose(2, 0, 1, 3)
            .reshape(128, 16 * D)
        ).astype(ml_dtypes.float8_e4m3fn)
        B1R[e] = np.broadcast_to(b1c, (128, F)).astype(BF16)
        if general_ln:
            GR[e] = np.broadcast_to(ln_g[e], (128, F)).astype(BF16)
            BR[e] = np.broadcast_to(ln_b[e], (128, F)).astype(BF16)

    in_maps = []
    scatter = []  # per core: (token_ids, valid, expert_row)
    for slot_a, slot_b in core_slots:
        tok_ids = np.zeros((tpc, PTILE), np.int64)
        valid = np.zeros((tpc, PTILE), bool)
        e_tile = np.zeros(tpc, np.int64)
        ti = 0
        for slot, size in ((slot_a, s1), (slot_b, s2)):
            e = slot["expert"]
            tiles, vmask = exp_tiles.get(e, (None, None))
            for k in range(size):
                if k < slot["nreal"]:
                    idx = cursor[e]
                    cursor[e] += 1
                    tok_ids[ti] = tiles[idx]
                    valid[ti] = vmask[idx]
                else:
                    tok_ids[ti] = tiles[0] if tiles is not None else 0
                    valid[ti] = False
                e_tile[ti] = e
                ti += 1
        ids = tok_ids.reshape(-1)
        xg = xr[ids]  # [tpc*128, D]
        xtt = (
            xg.reshape(tpc, PTILE, 8, 128)
            .transpose(0, 3, 2, 1)
            .reshape(tpc, 128, 8 * 128)
        ).astype(BF16)
        im = {
            "xtt": np.ascontiguousarray(xtt),
            "w1": np.stack([W1P[slot_a["expert"]], W1P[slot_b["expert"]]]),
            "w2": np.stack([W2P[slot_a["expert"]], W2P[slot_b["expert"]]]),
            "b1r": np.stack([B1R[slot_a["expert"]], B1R[slot_b["expert"]]]),
        }
        if general_ln:
            im["gr"] = np.stack([GR[slot_a["expert"]], GR[slot_b["expert"]]])
            im["br"] = np.stack([BR[slot_a["expert"]], BR[slot_b["expert"]]])
        in_maps.append(im)
        scatter.append((ids, valid.reshape(-1), np.repeat(e_tile, PTILE)))

    key = (tpc, s1, s2, general_ln)
    if key not in _PROG_CACHE:
        _PROG_CACHE[key] = _build_program(*key)
    nc = _PROG_CACHE[key]

    from concourse.bass_utils import run_bass_kernel_spmd

    LAST_CALL = (nc, in_maps)
    res = run_bass_kernel_spmd(nc, in_maps, core_ids=list(range(NCORES)))
    LAST_RESULT = res

    out = np.zeros((T, D), np.float32)
    covered = 0
    for core in range(NCORES):
        q = np.asarray(res.results[core]["out"]).reshape(
            tpc * PTILE, D).astype(np.float32)
        am = np.asarray(res.results[core]["sc"]).reshape(
            tpc * PTILE, 1).astype(np.float32)
        y = q * ((am + 1e-20) / (127.0 * 512.0))
        ids, valid, e_row = scatter[core]
        idv = ids[valid]
        ev = e_row[valid]
        wv = w[idv]
        alpha = res_scale[ev] * wv
        out[idv] = (y[valid] * alpha[:, None]
                    + xr[idv] * wv[:, None]
                    + alpha[:, None] * b2[ev])
        covered += int(valid.sum())
    assert covered == T, f"coverage {covered} != {T}"
    return out.reshape(B, S, D)

